# revision 24
# baseline (speedup 1.0000x reference)
"""JointRetention Trainium2 kernel.

out[b] = ((xpos(X_b Wq) xpos_down(X_b Wk)^T) * D[b%17]) @ (X_b Wv)

Strategy (v7):
  - Data-parallel over B*J=1088 across 8 cores (136 each; 136%17==0 so the
    joint index pattern is identical on every core).
  - bf16 everywhere (tolerance 2e-2; measured ~1e-3): halves DMA, enables
    FWL weight loads, and 2x DVE perf mode on all SBUF elementwise ops.
  - X host-packed TRANSPOSED (h on partitions) so the PE does zero
    transposes; host also packs xpos cos/sin tables, fused decay tables.
  - Even/odd d-permutation: xpos becomes elementwise muls + half combines.
  - Chunk-sparse scores/AV: D[i,j]=0 for j >= (i//81+1)*81, so score
    m-tile1 skips i<81 and AV accumulates only live (l, m) chunks.
  - Engine balance per quad (4 batches): PE ~9us (proj/V/S/AV matmuls),
    ACT (Q/K/V psum->sbuf bf16 casts + half the out drains), DVE (xpos
    muls at 2x, decay mask, 1 combine, half the out drains), GPSIMD
    (3 of 4 combines).
"""

import numpy as np
from ml_dtypes import bfloat16

L = 243
LP = 244                     # l padded to even for DVE 2x inner dim
H = 256
J = 17
NCORES = 8
NB = 1088
BPC = NB // NCORES           # 136 batches per core
NPAIR = BPC // 2             # 68 pairs per core
NOCT = NPAIR // 4            # 17 octs (4 pairs = 8 batches each) per core
SCALE_BASE = 512
CHUNK = 81

f32 = np.float32

_cache = {}


def _host_tables(W_Q, W_K, W_V, gamma):
    half = H // 2
    pe = np.arange(0, H, 2)
    po = np.arange(1, H, 2)
    Wcat = np.concatenate(
        [W_Q[:, pe], W_Q[:, po], W_K[:, pe], W_K[:, po], W_V], axis=1).astype(f32)
    W_all = np.stack([Wcat[0:128], Wcat[128:256]], axis=0)  # (2,128,768)

    base_scale = ((np.arange(0, H, 2, dtype=f32) + 0.4 * H) / (1.4 * H)).astype(f32)
    pos = np.arange(L, dtype=f32)
    scale = base_scale[None, :] ** (pos / SCALE_BASE)[:, None]
    inv_freq = (1.0 / 10000.0 ** (np.arange(half, dtype=f32) / half)).astype(f32)
    sinus = pos[:, None] * inv_freq[None, :]
    sin, cos = np.sin(sinus).astype(f32), np.cos(sinus).astype(f32)
    hCq = (cos * scale).T
    hSq = (sin * scale).T
    hCk = (cos / scale).T
    hSk = (sin / scale).T

    def padl(t):
        out = np.zeros((128, LP), f32)
        out[:, :L] = t
        return out

    def oct_tab(c, s):
        cp = np.concatenate([padl(c), padl(c)], axis=1)
        sp = np.concatenate([padl(s), padl(s)], axis=1)
        return np.concatenate([cp, sp] * 4, axis=1)       # (128, 3904)

    T_all = np.stack([oct_tab(hCq, hSq), oct_tab(hSq, hCq),
                      oct_tab(hCk, hSk), oct_tab(hSk, hCk)], axis=0)

    g = gamma.astype(f32)
    i = np.arange(L)[:, None]
    jj = np.arange(L)[None, :]
    allowed = jj < (i // CHUNK + 1) * CHUNK
    absd = np.abs(i - jj).astype(f32)
    D = g[:, None, None] ** absd[None]
    D = np.where(allowed[None], D, 0.0)
    D = np.where(np.isnan(D), 0.0, D).astype(f32)
    DTab = np.zeros((18, 128, 405), f32)
    for s in range(18):
        jt = s % J
        DTab[s, :, 0:L] = D[jt].T[0:128, :]
        DTab[s, 0:L - 128, L:405] = D[jt].T[128:L, 81:L]
    return (W_all.astype(bfloat16), T_all.astype(bfloat16),
            DTab.astype(bfloat16))


def _host_pack_x(Xc):
    # (BPC, 243, 256) f32 -> (NPAIR, 128, 1024) bf16, cols = hc*512+b*256+l
    Xp = Xc.reshape(NPAIR, 2, L, 2, 128)                  # pair, b, l, hc, p
    Xp = np.transpose(Xp, (0, 4, 3, 1, 2))                # pair, p, hc, b, l
    out = np.zeros((NPAIR, 128, 2, 2, 256), f32)
    out[:, :, :, :, 0:L] = Xp
    return np.ascontiguousarray(out.reshape(NPAIR, 128, 1024)).astype(bfloat16)


def _host_unpack_o(Oc):
    # (NPAIR, 128, 972) cols = b*486 + ht*243 + l -> (BPC, 243, 256) f32
    Op = Oc.astype(f32).reshape(NPAIR, 128, 2, 2, L)      # pair, p, b, ht, l
    Op = np.transpose(Op, (0, 2, 4, 3, 1))                # pair, b, l, ht, p
    return np.ascontiguousarray(Op.reshape(BPC, L, H))


def _build():
    import concourse.bacc as bacc
    import concourse.mybir as mybir
    from concourse import tile

    dt = mybir.dt
    F32 = dt.float32
    BF16 = dt.bfloat16
    MULT = mybir.AluOpType.mult
    ADD = mybir.AluOpType.add
    SUB = mybir.AluOpType.subtract

    nc = bacc.Bacc("TRN2", target_bir_lowering=False, debug=False,
                   num_devices=NCORES)
    X_d = nc.dram_tensor("X", (NPAIR, 128, 1024), BF16, kind="ExternalInput").ap()
    W_d = nc.dram_tensor("WALL", (2, 128, 768), BF16, kind="ExternalInput").ap()
    T_d = nc.dram_tensor("TTAB", (4, 128, 3904), BF16, kind="ExternalInput").ap()
    DT_d = nc.dram_tensor("DTAB", (18, 128, 405), BF16, kind="ExternalInput").ap()
    O_d = nc.dram_tensor("OUT", (NPAIR, 128, 972), BF16, kind="ExternalOutput").ap()

    with tile.TileContext(nc) as tc:
        with (
            tc.tile_pool(name="const", bufs=1) as const,
            tc.tile_pool(name="xin", bufs=6) as xin,
            tc.tile_pool(name="eo", bufs=2) as eo_pool,
            tc.tile_pool(name="tab", bufs=1) as tab_pool,
            tc.tile_pool(name="qx", bufs=2) as qx_pool,
            tc.tile_pool(name="vat", bufs=3) as vat,
            tc.tile_pool(name="osb", bufs=3) as osb_pool,
            tc.tile_pool(name="pqk", bufs=2, space="PSUM") as pqk,
            tc.tile_pool(name="pattn", bufs=2, space="PSUM") as pattn,
            tc.tile_pool(name="pout", bufs=1, space="PSUM") as pout,
        ):
            # ---- constants ----
            w_sb = [const.tile([128, 768], BF16, name=f"w{h}", tag=f"w{h}")
                    for h in range(2)]
            t_sb = [const.tile([128, 3904], BF16, name=f"t{i}", tag=f"t{i}")
                    for i in range(4)]
            dt_sb = const.tile([128, 18 * 405], BF16, name="dtab", tag="dtab")
            for h in range(2):
                nc.sync.dma_start(w_sb[h][:], W_d[h])
            for i in range(4):
                nc.sync.dma_start(t_sb[i][:], T_d[i])
            for s in range(18):
                nc.sync.dma_start(dt_sb[:, s * 405:(s + 1) * 405], DT_d[s])

            def load_x(t):
                xi = xin.tile([128, 1024], BF16, name="xi", tag="xi")
                nc.sync.dma_start(xi[:], X_d[t])
                return xi

            xt_cur = load_x(0)

            for o in range(NOCT):
                qeo = eo_pool.tile([128, 3904], BF16, name="qeo", tag="qeo")
                keo = eo_pool.tile([128, 3904], BF16, name="keo", tag="keo")
                vsb = [None, None, None, None]
                for hh in range(4):
                    t = 4 * o + hh
                    xt = xt_cur

                    # ---- proj Q,K: psum [128,1024], e block 0:488, o 512:1000
                    pq_t = pqk.tile([128, 1024], F32, name="pq", tag="pqk")
                    pk_t = pqk.tile([128, 1024], F32, name="pk", tag="pqk")
                    for ti, ps in ((0, pq_t), (1, pk_t)):
                        for eo in range(2):
                            for hc in range(2):
                                mov = xt[:, hc * 512:hc * 512 + 512].rearrange(
                                    "p (b l) -> p b l", b=2)[:, :, 0:LP]
                                nc.tensor.matmul(
                                    ps[:, eo * 512:eo * 512 + 488],
                                    w_sb[hc][:, (ti * 2 + eo) * 128:
                                             (ti * 2 + eo) * 128 + 128],
                                    mov,
                                    start=(hc == 0), stop=(hc == 1),
                                )

                    # ---- V: psum [128,1024] = [b0 m0 | b0 m1 | b1 m0 | b1 m1]
                    pv_t = pqk.tile([128, 1024], F32, name="pv", tag="pqk")
                    for b in range(2):
                        for mc in range(2):
                            for hc in range(2):
                                off = hc * 512 + b * 256 + mc * 128
                                nc.tensor.matmul(
                                    pv_t[:, b * 512 + mc * 256:
                                         b * 512 + mc * 256 + 256],
                                    xt[:, off:off + 128],
                                    w_sb[hc][:, 512:768],
                                    start=(hc == 0), stop=(hc == 1),
                                )

                    # prefetch next pair's X
                    if t + 1 < NPAIR:
                        xt_cur = load_x(t + 1)

                    # ---- ACT drains (psum f32 -> sbuf bf16)
                    src = pq_t[:].rearrange("p (e c) -> p e c", e=2)[:, :, 0:488]
                    dst = qeo[:, hh * 976:hh * 976 + 976].rearrange(
                        "p (e c) -> p e c", e=2)
                    nc.scalar.copy(dst, src)
                    src = pk_t[:].rearrange("p (e c) -> p e c", e=2)[:, :, 0:488]
                    dst = keo[:, hh * 976:hh * 976 + 976].rearrange(
                        "p (e c) -> p e c", e=2)
                    nc.scalar.copy(dst, src)
                    vt = vat.tile([128, 1024], BF16, name="vsb", tag=f"vsb{hh}")
                    nc.scalar.copy(vt[:], pv_t[:])
                    vsb[hh] = vt

                # ---- xpos muls (DVE, bf16 2x) ----
                ta_q = tab_pool.tile([128, 3904], BF16, name="taq", tag="taq")
                tb_q = tab_pool.tile([128, 3904], BF16, name="tbq", tag="tbq")
                ta_k = tab_pool.tile([128, 3904], BF16, name="tak", tag="tak")
                tb_k = tab_pool.tile([128, 3904], BF16, name="tbk", tag="tbk")
                nc.vector.tensor_tensor(ta_q[:], qeo[:], t_sb[0][:], MULT)
                nc.vector.tensor_tensor(tb_q[:], qeo[:], t_sb[1][:], MULT)
                nc.vector.tensor_tensor(ta_k[:], keo[:], t_sb[2][:], MULT)
                nc.vector.tensor_tensor(tb_k[:], keo[:], t_sb[3][:], MULT)

                # ---- combines -> qx/kx [128,2048] 256-strided (pads stay 0)
                qx_e = qx_pool.tile([128, 2048], BF16, name="qxe", tag="qxe")
                qx_o = qx_pool.tile([128, 2048], BF16, name="qxo", tag="qxo")
                kx_e = qx_pool.tile([128, 2048], BF16, name="kxe", tag="kxe")
                kx_o = qx_pool.tile([128, 2048], BF16, name="kxo", tag="kxo")
                # l-pad columns of kx feed S stationary slices; keep them zero
                # (qx is only ever read as moving operand over real columns)
                for z in (kx_e, kx_o):
                    nc.gpsimd.memset(
                        z[:].rearrange("p (g l) -> p g l", g=8)[:, :, LP:256], 0.0)

                def c_src(tab, eo):
                    a = tab[:].rearrange("p (pr c) -> p pr c", pr=4)
                    a = a[:, :, eo * 488:(eo + 1) * 488]
                    return a.rearrange("p pr (b l) -> p pr b l", b=2)

                def c_dst(dst):
                    return dst[:].rearrange(
                        "p (pr b l) -> p pr b l", pr=4, b=2)[:, :, :, 0:LP]

                # all combines on DVE: gpsimd elementwise halves DVE
                # throughput via the shared SBUF port (measured)
                nc.vector.tensor_tensor(
                    c_dst(qx_e), c_src(ta_q, 0), c_src(ta_q, 1), SUB)
                nc.vector.tensor_tensor(
                    c_dst(qx_o), c_src(tb_q, 1), c_src(tb_q, 0), ADD)
                nc.vector.tensor_tensor(
                    c_dst(kx_e), c_src(ta_k, 0), c_src(ta_k, 1), SUB)
                nc.vector.tensor_tensor(
                    c_dst(kx_o), c_src(tb_k, 1), c_src(tb_k, 0), ADD)

                # ---- attention per batch ----
                osb = None
                po = None
                for b in range(8):
                    hp = b // 2
                    bl = b % 2
                    if bl == 0:
                        osb = osb_pool.tile([128, 972], BF16,
                                            name="ob", tag=f"ob{hp % 2}")
                        po = pout.tile([128, 1024], F32, name="ops", tag="outp")
                    boff = b * 256
                    # scores S^T [128, 405]: mt0 cols 0:243, mt1 243:405
                    ps = pattn.tile([128, 512], F32, name="sps", tag="attn")
                    nc.tensor.matmul(ps[:, 0:243],
                                     kx_e[:, boff:boff + 128],
                                     qx_e[:, boff:boff + 243],
                                     start=True, stop=False)
                    nc.tensor.matmul(ps[:, 0:243],
                                     kx_o[:, boff:boff + 128],
                                     qx_o[:, boff:boff + 243],
                                     start=False, stop=True)
                    nc.tensor.matmul(ps[:, 243:405],
                                     kx_e[:, boff + 128:boff + 256],
                                     qx_e[:, boff + 81:boff + 243],
                                     start=True, stop=False)
                    nc.tensor.matmul(ps[:, 243:405],
                                     kx_o[:, boff + 128:boff + 256],
                                     qx_o[:, boff + 81:boff + 243],
                                     start=False, stop=True)

                    # decay mask (DVE, psum 1x) -> A^T bf16
                    slot = (8 * o + b) % J
                    at = vat.tile([128, 416], BF16, name="at", tag=f"at{b % 2}")
                    nc.vector.tensor_tensor(
                        at[:, 0:405], ps[:, 0:405],
                        dt_sb[:, slot * 405:slot * 405 + 405], MULT)

                    # AV: out^T [128, 486] = [ht0 l 0:243 | ht1 l 0:243]
                    v = vsb[hp]
                    for ht in range(2):
                        lhs0 = v[:, bl * 512 + ht * 128:bl * 512 + ht * 128 + 128]
                        lhs1 = v[:, bl * 512 + 256 + ht * 128:
                                 bl * 512 + 256 + ht * 128 + 128]
                        base = bl * 512 + ht * 243
                        nc.tensor.matmul(po[:, base:base + 243],
                                         lhs0, at[:, 0:243],
                                         start=True, stop=False)
                        nc.tensor.matmul(po[:, base + 81:base + 243],
                                         lhs1, at[:, 243:405],
                                         start=False, stop=True)

                    if bl == 1:
                        # one merged out drain per pair on ACT
                        src = po[:].rearrange("p (b c) -> p b c", b=2)[:, :, 0:486]
                        dst = osb[:].rearrange("p (b c) -> p b c", b=2)
                        nc.scalar.copy(dst, src)
                        nc.sync.dma_start(O_d[4 * o + hp], osb[:])

    nc.compile()
    return nc


def _get_nc():
    if "nc" not in _cache:
        _cache["nc"] = _build()
    return _cache["nc"]


def _run(in_maps, trace=False):
    from concourse import bass_utils
    nc = _get_nc()
    return bass_utils.run_bass_kernel_spmd(
        nc, in_maps, core_ids=list(range(NCORES)), trace=trace)


def kernel(X, W_Q, W_K, W_V, gamma, _trace=False):
    X = np.asarray(X, f32)
    W_all, T_all, DTab = _host_tables(
        np.asarray(W_Q, f32), np.asarray(W_K, f32),
        np.asarray(W_V, f32), np.asarray(gamma, f32))

    in_maps = []
    for c in range(NCORES):
        in_maps.append({
            "X": _host_pack_x(X[c * BPC:(c + 1) * BPC]),
            "WALL": W_all, "TTAB": T_all, "DTAB": DTab,
        })
    res = _run(in_maps, trace=_trace)
    out = np.concatenate([_host_unpack_o(r["OUT"]) for r in res.results],
                         axis=0)
    if _trace:
        _cache["last_result"] = res
    return out.astype(f32)


# revision 28
# speedup vs baseline: 1.0533x; 1.0533x over previous
"""JointRetention Trainium2 kernel.

out[b] = ((xpos(X_b Wq) xpos_down(X_b Wk)^T) * D[b%17]) @ (X_b Wv)

Strategy (v7):
  - Data-parallel over B*J=1088 across 8 cores (136 each; 136%17==0 so the
    joint index pattern is identical on every core).
  - bf16 everywhere (tolerance 2e-2; measured ~1e-3): halves DMA, enables
    FWL weight loads, and 2x DVE perf mode on all SBUF elementwise ops.
  - X host-packed TRANSPOSED (h on partitions) so the PE does zero
    transposes; host also packs xpos cos/sin tables, fused decay tables.
  - Even/odd d-permutation: xpos becomes elementwise muls + half combines.
  - Chunk-sparse scores/AV: D[i,j]=0 for j >= (i//81+1)*81, so score
    m-tile1 skips i<81 and AV accumulates only live (l, m) chunks.
  - Engine balance per quad (4 batches): PE ~9us (proj/V/S/AV matmuls),
    ACT (Q/K/V psum->sbuf bf16 casts + half the out drains), DVE (xpos
    muls at 2x, decay mask, 1 combine, half the out drains), GPSIMD
    (3 of 4 combines).
"""

import numpy as np
from ml_dtypes import bfloat16

L = 243
LP = 244                     # l padded to even for DVE 2x inner dim
H = 256
J = 17
NCORES = 8
NB = 1088
BPC = NB // NCORES           # 136 batches per core
NPAIR = BPC // 2             # 68 pairs per core
NQUAD = NPAIR // 2           # 34 quads per core
SCALE_BASE = 512
CHUNK = 81

f32 = np.float32

_cache = {}


def _host_tables(W_Q, W_K, W_V, gamma):
    half = H // 2
    pe = np.arange(0, H, 2)
    po = np.arange(1, H, 2)
    Wcat = np.concatenate(
        [W_Q[:, pe], W_Q[:, po], W_K[:, pe], W_K[:, po], W_V], axis=1).astype(f32)
    W_all = np.stack([Wcat[0:128], Wcat[128:256]], axis=0)  # (2,128,768)

    base_scale = ((np.arange(0, H, 2, dtype=f32) + 0.4 * H) / (1.4 * H)).astype(f32)
    pos = np.arange(L, dtype=f32)
    scale = base_scale[None, :] ** (pos / SCALE_BASE)[:, None]
    inv_freq = (1.0 / 10000.0 ** (np.arange(half, dtype=f32) / half)).astype(f32)
    sinus = pos[:, None] * inv_freq[None, :]
    sin, cos = np.sin(sinus).astype(f32), np.cos(sinus).astype(f32)
    hCq = (cos * scale).T
    hSq = (sin * scale).T
    hCk = (cos / scale).T
    hSk = (sin / scale).T

    def padl(t):
        out = np.zeros((128, LP), f32)
        out[:, :L] = t
        return out

    def quad_tab(c, s):
        cp = np.concatenate([padl(c), padl(c)], axis=1)
        sp = np.concatenate([padl(s), padl(s)], axis=1)
        return np.concatenate([cp, sp, cp, sp], axis=1)   # (128, 1952)

    T_all = np.stack([quad_tab(hCq, hSq), quad_tab(hSq, hCq),
                      quad_tab(hCk, hSk), quad_tab(hSk, hCk)], axis=0)

    g = gamma.astype(f32)
    i = np.arange(L)[:, None]
    jj = np.arange(L)[None, :]
    allowed = jj < (i // CHUNK + 1) * CHUNK
    absd = np.abs(i - jj).astype(f32)
    D = g[:, None, None] ** absd[None]
    D = np.where(allowed[None], D, 0.0)
    D = np.where(np.isnan(D), 0.0, D).astype(f32)
    DTab = np.zeros((18, 128, 405), f32)
    for s in range(18):
        jt = s % J
        DTab[s, :, 0:L] = D[jt].T[0:128, :]
        DTab[s, 0:L - 128, L:405] = D[jt].T[128:L, 81:L]
    return (W_all.astype(bfloat16), T_all.astype(bfloat16),
            DTab.astype(bfloat16))


def _host_pack_x(Xc):
    # (BPC, 243, 256) f32 -> (NPAIR, 128, 1024) bf16, cols = hc*512+b*256+l
    Xp = Xc.reshape(NPAIR, 2, L, 2, 128)                  # pair, b, l, hc, p
    Xp = np.transpose(Xp, (0, 4, 3, 1, 2))                # pair, p, hc, b, l
    out = np.zeros((NPAIR, 128, 2, 2, 256), f32)
    out[:, :, :, :, 0:L] = Xp
    return np.ascontiguousarray(out.reshape(NPAIR, 128, 1024)).astype(bfloat16)


def _host_unpack_o(Oc):
    # (NPAIR, 128, 972) cols = b*486 + ht*243 + l -> (BPC, 243, 256) f32
    Op = Oc.astype(f32).reshape(NPAIR, 128, 2, 2, L)      # pair, p, b, ht, l
    Op = np.transpose(Op, (0, 2, 4, 3, 1))                # pair, b, l, ht, p
    return np.ascontiguousarray(Op.reshape(BPC, L, H))


def _build():
    import concourse.bacc as bacc
    import concourse.mybir as mybir
    from concourse import tile

    dt = mybir.dt
    F32 = dt.float32
    BF16 = dt.bfloat16
    MULT = mybir.AluOpType.mult
    ADD = mybir.AluOpType.add
    SUB = mybir.AluOpType.subtract

    nc = bacc.Bacc("TRN2", target_bir_lowering=False, debug=False,
                   num_devices=NCORES)
    X_d = nc.dram_tensor("X", (NPAIR, 128, 1024), BF16, kind="ExternalInput").ap()
    W_d = nc.dram_tensor("WALL", (2, 128, 768), BF16, kind="ExternalInput").ap()
    T_d = nc.dram_tensor("TTAB", (4, 128, 1952), BF16, kind="ExternalInput").ap()
    DT_d = nc.dram_tensor("DTAB", (18, 128, 405), BF16, kind="ExternalInput").ap()
    O_d = nc.dram_tensor("OUT", (NPAIR, 128, 972), BF16, kind="ExternalOutput").ap()

    with tile.TileContext(nc) as tc:
        with (
            tc.tile_pool(name="const", bufs=1) as const,
            tc.tile_pool(name="xin", bufs=6) as xin,
            tc.tile_pool(name="eo", bufs=3) as eo_pool,
            tc.tile_pool(name="tab", bufs=3) as tab_pool,
            tc.tile_pool(name="qx", bufs=3) as qx_pool,
            tc.tile_pool(name="vat", bufs=4) as vat,
            tc.tile_pool(name="osb", bufs=4) as osb_pool,
            tc.tile_pool(name="pqk", bufs=2, space="PSUM") as pqk,
            tc.tile_pool(name="pattn", bufs=2, space="PSUM") as pattn,
            tc.tile_pool(name="pout", bufs=1, space="PSUM") as pout,
        ):
            # ---- constants ----
            w_sb = [const.tile([128, 768], BF16, name=f"w{h}", tag=f"w{h}")
                    for h in range(2)]
            t_sb = [const.tile([128, 1952], BF16, name=f"t{i}", tag=f"t{i}")
                    for i in range(4)]
            dt_sb = const.tile([128, 18 * 405], BF16, name="dtab", tag="dtab")
            for h in range(2):
                nc.sync.dma_start(w_sb[h][:], W_d[h])
            for i in range(4):
                nc.sync.dma_start(t_sb[i][:], T_d[i])
            for s in range(18):
                nc.sync.dma_start(dt_sb[:, s * 405:(s + 1) * 405], DT_d[s])

            def load_x(t):
                xi = xin.tile([128, 1024], BF16, name="xi", tag="xi")
                nc.sync.dma_start(xi[:], X_d[t])
                return xi

            xt_cur = load_x(0)

            for q in range(NQUAD):
                qeo = eo_pool.tile([128, 1952], BF16, name="qeo", tag="qeo")
                keo = eo_pool.tile([128, 1952], BF16, name="keo", tag="keo")
                vsb = [None, None]
                for hh in range(2):
                    t = 2 * q + hh
                    xt = xt_cur

                    # ---- proj Q,K: psum [128,1024], e block 0:488, o 512:1000
                    pq_t = pqk.tile([128, 1024], F32, name="pq", tag="pqk")
                    pk_t = pqk.tile([128, 1024], F32, name="pk", tag="pqk")
                    for ti, ps in ((0, pq_t), (1, pk_t)):
                        for eo in range(2):
                            for hc in range(2):
                                mov = xt[:, hc * 512:hc * 512 + 512].rearrange(
                                    "p (b l) -> p b l", b=2)[:, :, 0:LP]
                                nc.tensor.matmul(
                                    ps[:, eo * 512:eo * 512 + 488],
                                    w_sb[hc][:, (ti * 2 + eo) * 128:
                                             (ti * 2 + eo) * 128 + 128],
                                    mov,
                                    start=(hc == 0), stop=(hc == 1),
                                )

                    # ---- V: psum [128,1024] = [b0 m0 | b0 m1 | b1 m0 | b1 m1]
                    pv_t = pqk.tile([128, 1024], F32, name="pv", tag="pqk")
                    for b in range(2):
                        for mc in range(2):
                            for hc in range(2):
                                off = hc * 512 + b * 256 + mc * 128
                                nc.tensor.matmul(
                                    pv_t[:, b * 512 + mc * 256:
                                         b * 512 + mc * 256 + 256],
                                    xt[:, off:off + 128],
                                    w_sb[hc][:, 512:768],
                                    start=(hc == 0), stop=(hc == 1),
                                )

                    # prefetch next pair's X
                    if t + 1 < NPAIR:
                        xt_cur = load_x(t + 1)

                    # ---- ACT drains (psum f32 -> sbuf bf16)
                    src = pq_t[:].rearrange("p (e c) -> p e c", e=2)[:, :, 0:488]
                    dst = qeo[:, hh * 976:hh * 976 + 976].rearrange(
                        "p (e c) -> p e c", e=2)
                    nc.scalar.copy(dst, src)
                    src = pk_t[:].rearrange("p (e c) -> p e c", e=2)[:, :, 0:488]
                    dst = keo[:, hh * 976:hh * 976 + 976].rearrange(
                        "p (e c) -> p e c", e=2)
                    nc.scalar.copy(dst, src)
                    vt = vat.tile([128, 1024], BF16, name="vsb", tag=f"vsb{hh}")
                    nc.scalar.copy(vt[:], pv_t[:])
                    vsb[hh] = vt

                # ---- xpos muls (DVE, bf16 2x) ----
                ta_q = tab_pool.tile([128, 1952], BF16, name="taq", tag="taq")
                tb_q = tab_pool.tile([128, 1952], BF16, name="tbq", tag="tbq")
                ta_k = tab_pool.tile([128, 1952], BF16, name="tak", tag="tak")
                tb_k = tab_pool.tile([128, 1952], BF16, name="tbk", tag="tbk")
                nc.vector.tensor_tensor(ta_q[:], qeo[:], t_sb[0][:], MULT)
                nc.vector.tensor_tensor(tb_q[:], qeo[:], t_sb[1][:], MULT)
                nc.vector.tensor_tensor(ta_k[:], keo[:], t_sb[2][:], MULT)
                nc.vector.tensor_tensor(tb_k[:], keo[:], t_sb[3][:], MULT)

                # ---- combines -> qx/kx [128,1024] 256-strided (pads stay 0)
                qx_e = qx_pool.tile([128, 1024], BF16, name="qxe", tag="qxe")
                qx_o = qx_pool.tile([128, 1024], BF16, name="qxo", tag="qxo")
                kx_e = qx_pool.tile([128, 1024], BF16, name="kxe", tag="kxe")
                kx_o = qx_pool.tile([128, 1024], BF16, name="kxo", tag="kxo")
                # (no pad zeroing needed: S mt1 stationary reads only the 115
                # real columns, and qx is only read as moving over real cols)

                def c_src(tab, eo):
                    a = tab[:].rearrange("p (pr c) -> p pr c", pr=2)
                    a = a[:, :, eo * 488:(eo + 1) * 488]
                    return a.rearrange("p pr (b l) -> p pr b l", b=2)

                def c_dst(dst):
                    return dst[:].rearrange(
                        "p (pr b l) -> p pr b l", pr=2, b=2)[:, :, :, 0:LP]

                # all combines on DVE: gpsimd elementwise halves DVE
                # throughput via the shared SBUF port (measured)
                nc.vector.tensor_tensor(
                    c_dst(qx_e), c_src(ta_q, 0), c_src(ta_q, 1), SUB)
                nc.vector.tensor_tensor(
                    c_dst(qx_o), c_src(tb_q, 1), c_src(tb_q, 0), ADD)
                nc.vector.tensor_tensor(
                    c_dst(kx_e), c_src(ta_k, 0), c_src(ta_k, 1), SUB)
                nc.vector.tensor_tensor(
                    c_dst(kx_o), c_src(tb_k, 1), c_src(tb_k, 0), ADD)

                # ---- attention per batch ----
                osb = [None, None]
                po = None
                for b in range(4):
                    hh = b // 2
                    bl = b % 2
                    if bl == 0:
                        osb[hh] = osb_pool.tile([128, 972], BF16,
                                                name="ob", tag=f"ob{hh}")
                        po = pout.tile([128, 1024], F32, name="ops", tag="outp")
                    boff = b * 256
                    # scores S^T [128, 405]: mt0 cols 0:243, mt1 243:405
                    ps = pattn.tile([128, 512], F32, name="sps", tag="attn")
                    nc.tensor.matmul(ps[:, 0:243],
                                     kx_e[:, boff:boff + 128],
                                     qx_e[:, boff:boff + 243],
                                     start=True, stop=False)
                    nc.tensor.matmul(ps[:, 0:243],
                                     kx_o[:, boff:boff + 128],
                                     qx_o[:, boff:boff + 243],
                                     start=False, stop=True)
                    nc.tensor.matmul(ps[0:115, 243:405],
                                     kx_e[:, boff + 128:boff + 243],
                                     qx_e[:, boff + 81:boff + 243],
                                     start=True, stop=False)
                    nc.tensor.matmul(ps[0:115, 243:405],
                                     kx_o[:, boff + 128:boff + 243],
                                     qx_o[:, boff + 81:boff + 243],
                                     start=False, stop=True)

                    # decay mask (DVE, psum 1x) -> A^T bf16
                    slot = (4 * q + b) % J
                    at = vat.tile([128, 416], BF16, name="at", tag=f"at{b % 2}")
                    nc.vector.tensor_tensor(
                        at[:, 0:405], ps[:, 0:405],
                        dt_sb[:, slot * 405:slot * 405 + 405], MULT)

                    # AV: out^T [128, 486] = [ht0 l 0:243 | ht1 l 0:243]
                    v = vsb[hh]
                    for ht in range(2):
                        lhs0 = v[:, bl * 512 + ht * 128:bl * 512 + ht * 128 + 128]
                        # mt1: only 115 real m rows — rows 115:128 of the S/A
                        # mt1 region are never written (stale) and never read
                        lhs1 = v[0:115, bl * 512 + 256 + ht * 128:
                                 bl * 512 + 256 + ht * 128 + 128]
                        base = bl * 512 + ht * 243
                        nc.tensor.matmul(po[:, base:base + 243],
                                         lhs0, at[:, 0:243],
                                         start=True, stop=False)
                        nc.tensor.matmul(po[:, base + 81:base + 243],
                                         lhs1, at[0:115, 243:405],
                                         start=False, stop=True)

                    if bl == 1:
                        # one merged out drain per pair on ACT
                        src = po[:].rearrange("p (b c) -> p b c", b=2)[:, :, 0:486]
                        dst = osb[hh][:].rearrange("p (b c) -> p b c", b=2)
                        nc.scalar.copy(dst, src)
                        nc.sync.dma_start(O_d[2 * q + hh], osb[hh][:])

    nc.compile()
    return nc


def _get_nc():
    if "nc" not in _cache:
        _cache["nc"] = _build()
    return _cache["nc"]


def _run(in_maps, trace=False):
    from concourse import bass_utils
    nc = _get_nc()
    return bass_utils.run_bass_kernel_spmd(
        nc, in_maps, core_ids=list(range(NCORES)), trace=trace)


def kernel(X, W_Q, W_K, W_V, gamma, _trace=False):
    X = np.asarray(X, f32)
    W_all, T_all, DTab = _host_tables(
        np.asarray(W_Q, f32), np.asarray(W_K, f32),
        np.asarray(W_V, f32), np.asarray(gamma, f32))

    in_maps = []
    for c in range(NCORES):
        in_maps.append({
            "X": _host_pack_x(X[c * BPC:(c + 1) * BPC]),
            "WALL": W_all, "TTAB": T_all, "DTAB": DTab,
        })
    res = _run(in_maps, trace=_trace)
    out = np.concatenate([_host_unpack_o(r["OUT"]) for r in res.results],
                         axis=0)
    if _trace:
        _cache["last_result"] = res
    return out.astype(f32)


# revision 35
# speedup vs baseline: 1.1189x; 1.0623x over previous
"""JointRetention Trainium2 kernel.

out[b] = ((xpos(X_b Wq) xpos_down(X_b Wk)^T) * D[b%17]) @ (X_b Wv)

Strategy (v7):
  - Data-parallel over B*J=1088 across 8 cores (136 each; 136%17==0 so the
    joint index pattern is identical on every core).
  - bf16 everywhere (tolerance 2e-2; measured ~1e-3): halves DMA, enables
    FWL weight loads, and 2x DVE perf mode on all SBUF elementwise ops.
  - X host-packed TRANSPOSED (h on partitions) so the PE does zero
    transposes; host also packs xpos cos/sin tables, fused decay tables.
  - Even/odd d-permutation: xpos becomes elementwise muls + half combines.
  - Chunk-sparse scores/AV: D[i,j]=0 for j >= (i//81+1)*81, so score
    m-tile1 skips i<81 and AV accumulates only live (l, m) chunks.
  - Engine balance per quad (4 batches): PE ~9us (proj/V/S/AV matmuls),
    ACT (Q/K/V psum->sbuf bf16 casts + half the out drains), DVE (xpos
    muls at 2x, decay mask, 1 combine, half the out drains), GPSIMD
    (3 of 4 combines).
"""

import numpy as np
from ml_dtypes import bfloat16

L = 243
LP = 244                     # l padded to even for DVE 2x inner dim
H = 256
J = 17
NCORES = 8
NB = 1088
BPC = NB // NCORES           # 136 batches per core
NPAIR = BPC // 2             # 68 pairs per core
NQUAD = NPAIR // 2           # 34 quads per core
SCALE_BASE = 512
CHUNK = 81

f32 = np.float32

_cache = {}


def _host_tables(W_Q, W_K, W_V, gamma):
    half = H // 2
    pe = np.arange(0, H, 2)
    po = np.arange(1, H, 2)
    Wcat = np.concatenate(
        [W_Q[:, pe], W_Q[:, po], W_K[:, pe], W_K[:, po], W_V], axis=1).astype(f32)
    W_all = np.stack([Wcat[0:128], Wcat[128:256]], axis=0)  # (2,128,768)

    base_scale = ((np.arange(0, H, 2, dtype=f32) + 0.4 * H) / (1.4 * H)).astype(f32)
    pos = np.arange(L, dtype=f32)
    scale = base_scale[None, :] ** (pos / SCALE_BASE)[:, None]
    inv_freq = (1.0 / 10000.0 ** (np.arange(half, dtype=f32) / half)).astype(f32)
    sinus = pos[:, None] * inv_freq[None, :]
    sin, cos = np.sin(sinus).astype(f32), np.cos(sinus).astype(f32)
    hCq = (cos * scale).T
    hSq = (sin * scale).T
    hCk = (cos / scale).T
    hSk = (sin / scale).T

    def padl(t):
        out = np.zeros((128, LP), f32)
        out[:, :L] = t
        return out

    def fused_tab(c, s):
        # per pair-block (1952): rep0 = [C|S] (-> TA = [QeC|QoS]),
        # rep1 = [S|C] (-> TB = [QeS|QoC]); x2 pairs
        cp = np.concatenate([padl(c), padl(c)], axis=1)
        sp = np.concatenate([padl(s), padl(s)], axis=1)
        blk = np.concatenate([cp, sp, sp, cp], axis=1)    # (128, 1952)
        return np.concatenate([blk, blk], axis=1)         # (128, 3904)

    T_all = np.stack([fused_tab(hCq, hSq), fused_tab(hCk, hSk)], axis=0)

    g = gamma.astype(f32)
    i = np.arange(L)[:, None]
    jj = np.arange(L)[None, :]
    allowed = jj < (i // CHUNK + 1) * CHUNK
    absd = np.abs(i - jj).astype(f32)
    D = g[:, None, None] ** absd[None]
    D = np.where(allowed[None], D, 0.0)
    D = np.where(np.isnan(D), 0.0, D).astype(f32)
    DTab = np.zeros((18, 128, 405), f32)
    for s in range(18):
        jt = s % J
        DTab[s, :, 0:L] = D[jt].T[0:128, :]
        DTab[s, 0:L - 128, L:405] = D[jt].T[128:L, 81:L]
    return (W_all.astype(bfloat16), T_all.astype(bfloat16),
            DTab.astype(bfloat16))


def _host_pack_x(Xc):
    # (BPC, 243, 256) f32 -> (NPAIR, 128, 1024) bf16, cols = hc*512+b*256+l
    Xp = Xc.reshape(NPAIR, 2, L, 2, 128)                  # pair, b, l, hc, p
    Xp = np.transpose(Xp, (0, 4, 3, 1, 2))                # pair, p, hc, b, l
    out = np.zeros((NPAIR, 128, 2, 2, 256), f32)
    out[:, :, :, :, 0:L] = Xp
    return np.ascontiguousarray(out.reshape(NPAIR, 128, 1024)).astype(bfloat16)


def _host_unpack_o(Oc):
    # (NPAIR, 128, 972) cols = b*486 + ht*243 + l -> (BPC, 243, 256) f32
    Op = Oc.astype(f32).reshape(NPAIR, 128, 2, 2, L)      # pair, p, b, ht, l
    Op = np.transpose(Op, (0, 2, 4, 3, 1))                # pair, b, l, ht, p
    return np.ascontiguousarray(Op.reshape(BPC, L, H))


def _build():
    import concourse.bacc as bacc
    import concourse.mybir as mybir
    from concourse import tile

    dt = mybir.dt
    F32 = dt.float32
    BF16 = dt.bfloat16
    MULT = mybir.AluOpType.mult
    ADD = mybir.AluOpType.add
    SUB = mybir.AluOpType.subtract

    nc = bacc.Bacc("TRN2", target_bir_lowering=False, debug=False,
                   num_devices=NCORES)
    X_d = nc.dram_tensor("X", (NPAIR, 128, 1024), BF16, kind="ExternalInput").ap()
    W_d = nc.dram_tensor("WALL", (2, 128, 768), BF16, kind="ExternalInput").ap()
    T_d = nc.dram_tensor("TTAB", (2, 128, 3904), BF16, kind="ExternalInput").ap()
    DT_d = nc.dram_tensor("DTAB", (18, 128, 405), BF16, kind="ExternalInput").ap()
    O_d = nc.dram_tensor("OUT", (NPAIR, 128, 972), BF16, kind="ExternalOutput").ap()

    with tile.TileContext(nc) as tc:
        with (
            tc.tile_pool(name="const", bufs=1) as const,
            tc.tile_pool(name="xin", bufs=6) as xin,
            tc.tile_pool(name="eo", bufs=3) as eo_pool,
            tc.tile_pool(name="tab", bufs=3) as tab_pool,
            tc.tile_pool(name="qx", bufs=3) as qx_pool,
            tc.tile_pool(name="vat", bufs=4) as vat,
            tc.tile_pool(name="osb", bufs=4) as osb_pool,
            tc.tile_pool(name="pqk", bufs=2, space="PSUM") as pqk,
            tc.tile_pool(name="pattn", bufs=1, space="PSUM") as pattn,
            tc.tile_pool(name="pout", bufs=1, space="PSUM") as pout,
        ):
            # ---- constants ----
            w_sb = [const.tile([128, 768], BF16, name=f"w{h}", tag=f"w{h}")
                    for h in range(2)]
            t_sb = [const.tile([128, 3904], BF16, name=f"t{i}", tag=f"t{i}")
                    for i in range(2)]
            dt_sb = const.tile([128, 18 * 405], BF16, name="dtab", tag="dtab")
            for h in range(2):
                nc.sync.dma_start(w_sb[h][:], W_d[h])
            for i in range(2):
                nc.sync.dma_start(t_sb[i][:], T_d[i])
            for s in range(18):
                nc.sync.dma_start(dt_sb[:, s * 405:(s + 1) * 405], DT_d[s])

            def load_x(t):
                xi = xin.tile([128, 1024], BF16, name="xi", tag="xi")
                nc.sync.dma_start(xi[:], X_d[t])
                return xi

            xt_cur = load_x(0)

            for q in range(NQUAD):
                qeo = eo_pool.tile([128, 1952], BF16, name="qeo", tag="qeo")
                keo = eo_pool.tile([128, 1952], BF16, name="keo", tag="keo")
                vsb = [None, None]
                for hh in range(2):
                    t = 2 * q + hh
                    xt = xt_cur

                    # ---- proj Q,K: psum [128,1024], e block 0:488, o 512:1000
                    pq_t = pqk.tile([128, 1024], F32, name="pq", tag="pqk")
                    pk_t = pqk.tile([128, 1024], F32, name="pk", tag="pqk")
                    for ti, ps in ((0, pq_t), (1, pk_t)):
                        for eo in range(2):
                            for hc in range(2):
                                mov = xt[:, hc * 512:hc * 512 + 512].rearrange(
                                    "p (b l) -> p b l", b=2)[:, :, 0:LP]
                                nc.tensor.matmul(
                                    ps[:, eo * 512:eo * 512 + 488],
                                    w_sb[hc][:, (ti * 2 + eo) * 128:
                                             (ti * 2 + eo) * 128 + 128],
                                    mov,
                                    start=(hc == 0), stop=(hc == 1),
                                )

                    # ---- V: psum [128,1024] = [b0 m0 | b0 m1 | b1 m0 | b1 m1]
                    pv_t = pqk.tile([128, 1024], F32, name="pv", tag="pqk")
                    for b in range(2):
                        for mc in range(2):
                            for hc in range(2):
                                off = hc * 512 + b * 256 + mc * 128
                                nc.tensor.matmul(
                                    pv_t[:, b * 512 + mc * 256:
                                         b * 512 + mc * 256 + 256],
                                    xt[:, off:off + 128],
                                    w_sb[hc][:, 512:768],
                                    start=(hc == 0), stop=(hc == 1),
                                )

                    # prefetch next pair's X
                    if t + 1 < NPAIR:
                        xt_cur = load_x(t + 1)

                    # ---- ACT drains (psum f32 -> sbuf bf16)
                    src = pq_t[:].rearrange("p (e c) -> p e c", e=2)[:, :, 0:488]
                    dst = qeo[:, hh * 976:hh * 976 + 976].rearrange(
                        "p (e c) -> p e c", e=2)
                    nc.scalar.copy(dst, src)
                    src = pk_t[:].rearrange("p (e c) -> p e c", e=2)[:, :, 0:488]
                    dst = keo[:, hh * 976:hh * 976 + 976].rearrange(
                        "p (e c) -> p e c", e=2)
                    nc.scalar.copy(dst, src)
                    vt = vat.tile([128, 1024], BF16, name="vsb", tag=f"vsb{hh}")
                    nc.scalar.copy(vt[:], pv_t[:])
                    vsb[hh] = vt

                # ---- xpos muls (DVE, bf16 2x): one fused op per side reads
                # the projections twice via a stride-0 broadcast dim and
                # produces all four products [TA_p0|TB_p0|TA_p1|TB_p1]
                ta_q = tab_pool.tile([128, 3904], BF16, name="taq", tag="taq")
                ta_k = tab_pool.tile([128, 3904], BF16, name="tak", tag="tak")
                for src_eo, tab, dst in ((qeo, t_sb[0], ta_q),
                                         (keo, t_sb[1], ta_k)):
                    s = src_eo[:].rearrange("p (pr c) -> p pr c", pr=2)
                    s = s.unsqueeze(2).broadcast_to([128, 2, 2, 976])
                    nc.vector.tensor_tensor(
                        dst[:].rearrange("p (pr r c) -> p pr r c", pr=2, r=2),
                        s,
                        tab[:].rearrange("p (pr r c) -> p pr r c", pr=2, r=2),
                        MULT)

                # ---- combines -> qx/kx [128,1024] 256-strided (pads stay 0)
                qx_e = qx_pool.tile([128, 1024], BF16, name="qxe", tag="qxe")
                qx_o = qx_pool.tile([128, 1024], BF16, name="qxo", tag="qxo")
                kx_e = qx_pool.tile([128, 1024], BF16, name="kxe", tag="kxe")
                kx_o = qx_pool.tile([128, 1024], BF16, name="kxo", tag="kxo")
                # (no pad zeroing needed: S mt1 stationary reads only the 115
                # real columns, and qx is only read as moving over real cols)

                def c_src(tab, blk):
                    # 488-blocks per pair: 0=QeC, 1=QoS, 2=QeS, 3=QoC
                    a = tab[:].rearrange("p (pr c) -> p pr c", pr=2)
                    a = a[:, :, blk * 488:(blk + 1) * 488]
                    return a.rearrange("p pr (b l) -> p pr b l", b=2)

                def c_dst(dst):
                    return dst[:].rearrange(
                        "p (pr b l) -> p pr b l", pr=2, b=2)[:, :, :, 0:LP]

                # all combines on DVE: gpsimd elementwise halves DVE
                # throughput via the shared SBUF port (measured)
                nc.vector.tensor_tensor(
                    c_dst(qx_e), c_src(ta_q, 0), c_src(ta_q, 1), SUB)
                nc.vector.tensor_tensor(
                    c_dst(qx_o), c_src(ta_q, 3), c_src(ta_q, 2), ADD)
                nc.vector.tensor_tensor(
                    c_dst(kx_e), c_src(ta_k, 0), c_src(ta_k, 1), SUB)
                nc.vector.tensor_tensor(
                    c_dst(kx_o), c_src(ta_k, 3), c_src(ta_k, 2), ADD)

                # ---- attention per pair ----
                for hh in range(2):
                    t = 2 * q + hh
                    osb = osb_pool.tile([128, 972], BF16,
                                        name="ob", tag=f"ob{hh}")
                    po = pout.tile([128, 1024], F32, name="ops", tag="outp")
                    # scores for both batches in one 2-bank tile:
                    # batch bl at cols bl*512 + [mt0 0:243 | mt1 243:405]
                    ps = pattn.tile([128, 1024], F32, name="sps", tag="attn")
                    for bl in range(2):
                        boff = (2 * hh + bl) * 256
                        soff = bl * 512
                        nc.tensor.matmul(ps[:, soff:soff + 243],
                                         kx_e[:, boff:boff + 128],
                                         qx_e[:, boff:boff + 243],
                                         start=True, stop=False)
                        nc.tensor.matmul(ps[:, soff:soff + 243],
                                         kx_o[:, boff:boff + 128],
                                         qx_o[:, boff:boff + 243],
                                         start=False, stop=True)
                        nc.tensor.matmul(ps[0:115, soff + 243:soff + 405],
                                         kx_e[:, boff + 128:boff + 243],
                                         qx_e[:, boff + 81:boff + 243],
                                         start=True, stop=False)
                        nc.tensor.matmul(ps[0:115, soff + 243:soff + 405],
                                         kx_o[:, boff + 128:boff + 243],
                                         qx_o[:, boff + 81:boff + 243],
                                         start=False, stop=True)

                    # decay mask for the pair (DVE, psum 1x) -> A^T bf16;
                    # slots (2t)%17 and +1 are contiguous in the 18-slot table
                    s0 = (2 * t) % J
                    at = vat.tile([128, 832], BF16, name="at", tag=f"at{hh}")
                    nc.vector.tensor_tensor(
                        at[:].rearrange("p (k c) -> p k c", k=2)[:, :, 0:405],
                        ps[:].rearrange("p (k c) -> p k c", k=2)[:, :, 0:405],
                        dt_sb[:, s0 * 405:s0 * 405 + 810].rearrange(
                            "p (k c) -> p k c", k=2),
                        MULT)

                    # AV: out^T [128, 486] = [ht0 l 0:243 | ht1 l 0:243]
                    v = vsb[hh]
                    for bl in range(2):
                        aoff = bl * 416
                        for ht in range(2):
                            lhs0 = v[:, bl * 512 + ht * 128:
                                     bl * 512 + ht * 128 + 128]
                            # mt1: only 115 real m rows — rows 115:128 of the
                            # S/A mt1 region are never written and never read
                            lhs1 = v[0:115, bl * 512 + 256 + ht * 128:
                                     bl * 512 + 256 + ht * 128 + 128]
                            base = bl * 512 + ht * 243
                            nc.tensor.matmul(po[:, base:base + 243],
                                             lhs0, at[:, aoff:aoff + 243],
                                             start=True, stop=False)
                            nc.tensor.matmul(po[:, base + 81:base + 243],
                                             lhs1, at[0:115, aoff + 243:aoff + 405],
                                             start=False, stop=True)

                    # one merged out drain per pair on ACT
                    src = po[:].rearrange("p (b c) -> p b c", b=2)[:, :, 0:486]
                    dst = osb[:].rearrange("p (b c) -> p b c", b=2)
                    nc.scalar.copy(dst, src)
                    nc.sync.dma_start(O_d[t], osb[:])

    nc.compile()
    return nc


def _get_nc():
    if "nc" not in _cache:
        _cache["nc"] = _build()
    return _cache["nc"]


def _run(in_maps, trace=False):
    from concourse import bass_utils
    nc = _get_nc()
    return bass_utils.run_bass_kernel_spmd(
        nc, in_maps, core_ids=list(range(NCORES)), trace=trace)


def kernel(X, W_Q, W_K, W_V, gamma, _trace=False):
    X = np.asarray(X, f32)
    W_all, T_all, DTab = _host_tables(
        np.asarray(W_Q, f32), np.asarray(W_K, f32),
        np.asarray(W_V, f32), np.asarray(gamma, f32))

    in_maps = []
    for c in range(NCORES):
        in_maps.append({
            "X": _host_pack_x(X[c * BPC:(c + 1) * BPC]),
            "WALL": W_all, "TTAB": T_all, "DTAB": DTab,
        })
    res = _run(in_maps, trace=_trace)
    out = np.concatenate([_host_unpack_o(r["OUT"]) for r in res.results],
                         axis=0)
    if _trace:
        _cache["last_result"] = res
    return out.astype(f32)


# revision 36
# speedup vs baseline: 1.1338x; 1.0133x over previous
"""JointRetention Trainium2 kernel.

out[b] = ((xpos(X_b Wq) xpos_down(X_b Wk)^T) * D[b%17]) @ (X_b Wv)

Strategy (v7):
  - Data-parallel over B*J=1088 across 8 cores (136 each; 136%17==0 so the
    joint index pattern is identical on every core).
  - bf16 everywhere (tolerance 2e-2; measured ~1e-3): halves DMA, enables
    FWL weight loads, and 2x DVE perf mode on all SBUF elementwise ops.
  - X host-packed TRANSPOSED (h on partitions) so the PE does zero
    transposes; host also packs xpos cos/sin tables, fused decay tables.
  - Even/odd d-permutation: xpos becomes elementwise muls + half combines.
  - Chunk-sparse scores/AV: D[i,j]=0 for j >= (i//81+1)*81, so score
    m-tile1 skips i<81 and AV accumulates only live (l, m) chunks.
  - Engine balance per quad (4 batches): PE ~9us (proj/V/S/AV matmuls),
    ACT (Q/K/V psum->sbuf bf16 casts + half the out drains), DVE (xpos
    muls at 2x, decay mask, 1 combine, half the out drains), GPSIMD
    (3 of 4 combines).
"""

import numpy as np
from ml_dtypes import bfloat16

L = 243
LP = 244                     # l padded to even for DVE 2x inner dim
H = 256
J = 17
NCORES = 8
NB = 1088
BPC = NB // NCORES           # 136 batches per core
NPAIR = BPC // 2             # 68 pairs per core
NQUAD = NPAIR // 2           # 34 quads per core
SCALE_BASE = 512
CHUNK = 81

f32 = np.float32

_cache = {}


def _host_tables(W_Q, W_K, W_V, gamma):
    half = H // 2
    pe = np.arange(0, H, 2)
    po = np.arange(1, H, 2)
    Wcat = np.concatenate(
        [W_Q[:, pe], W_Q[:, po], W_K[:, pe], W_K[:, po], W_V], axis=1).astype(f32)
    W_all = np.stack([Wcat[0:128], Wcat[128:256]], axis=0)  # (2,128,768)

    base_scale = ((np.arange(0, H, 2, dtype=f32) + 0.4 * H) / (1.4 * H)).astype(f32)
    pos = np.arange(L, dtype=f32)
    scale = base_scale[None, :] ** (pos / SCALE_BASE)[:, None]
    inv_freq = (1.0 / 10000.0 ** (np.arange(half, dtype=f32) / half)).astype(f32)
    sinus = pos[:, None] * inv_freq[None, :]
    sin, cos = np.sin(sinus).astype(f32), np.cos(sinus).astype(f32)
    hCq = (cos * scale).T
    hSq = (sin * scale).T
    hCk = (cos / scale).T
    hSk = (sin / scale).T

    def padl(t):
        out = np.zeros((128, LP), f32)
        out[:, :L] = t
        return out

    def fused_tab(c, s):
        # per pair-block (1952): rep0 = [C|S] (-> TA = [QeC|QoS]),
        # rep1 = [S|C] (-> TB = [QeS|QoC]); x2 pairs
        cp = np.concatenate([padl(c), padl(c)], axis=1)
        sp = np.concatenate([padl(s), padl(s)], axis=1)
        blk = np.concatenate([cp, sp, sp, cp], axis=1)    # (128, 1952)
        return np.concatenate([blk, blk], axis=1)         # (128, 3904)

    T_all = np.stack([fused_tab(hCq, hSq), fused_tab(hCk, hSk)], axis=0)

    g = gamma.astype(f32)
    i = np.arange(L)[:, None]
    jj = np.arange(L)[None, :]
    allowed = jj < (i // CHUNK + 1) * CHUNK
    absd = np.abs(i - jj).astype(f32)
    D = g[:, None, None] ** absd[None]
    D = np.where(allowed[None], D, 0.0)
    D = np.where(np.isnan(D), 0.0, D).astype(f32)
    DTab = np.zeros((18, 128, 405), f32)
    for s in range(18):
        jt = s % J
        DTab[s, :, 0:L] = D[jt].T[0:128, :]
        DTab[s, 0:L - 128, L:405] = D[jt].T[128:L, 81:L]
    return (W_all.astype(bfloat16), T_all.astype(bfloat16),
            DTab.astype(bfloat16))


def _host_pack_x(Xc):
    # (BPC, 243, 256) f32 -> (NPAIR, 128, 1024) bf16, cols = hc*512+b*256+l
    Xp = Xc.reshape(NPAIR, 2, L, 2, 128)                  # pair, b, l, hc, p
    Xp = np.transpose(Xp, (0, 4, 3, 1, 2))                # pair, p, hc, b, l
    out = np.zeros((NPAIR, 128, 2, 2, 256), f32)
    out[:, :, :, :, 0:L] = Xp
    return np.ascontiguousarray(out.reshape(NPAIR, 128, 1024)).astype(bfloat16)


def _host_unpack_o(Oc):
    # (NPAIR, 128, 972) cols = b*486 + ht*243 + l -> (BPC, 243, 256) f32
    Op = Oc.astype(f32).reshape(NPAIR, 128, 2, 2, L)      # pair, p, b, ht, l
    Op = np.transpose(Op, (0, 2, 4, 3, 1))                # pair, b, l, ht, p
    return np.ascontiguousarray(Op.reshape(BPC, L, H))


def _build():
    import concourse.bacc as bacc
    import concourse.mybir as mybir
    from concourse import tile

    dt = mybir.dt
    F32 = dt.float32
    BF16 = dt.bfloat16
    MULT = mybir.AluOpType.mult
    ADD = mybir.AluOpType.add
    SUB = mybir.AluOpType.subtract

    nc = bacc.Bacc("TRN2", target_bir_lowering=False, debug=False,
                   num_devices=NCORES)
    X_d = nc.dram_tensor("X", (NPAIR, 128, 1024), BF16, kind="ExternalInput").ap()
    W_d = nc.dram_tensor("WALL", (2, 128, 768), BF16, kind="ExternalInput").ap()
    T_d = nc.dram_tensor("TTAB", (2, 128, 3904), BF16, kind="ExternalInput").ap()
    DT_d = nc.dram_tensor("DTAB", (18, 128, 405), BF16, kind="ExternalInput").ap()
    O_d = nc.dram_tensor("OUT", (NPAIR, 128, 972), BF16, kind="ExternalOutput").ap()

    with tile.TileContext(nc) as tc:
        with (
            tc.tile_pool(name="const", bufs=1) as const,
            tc.tile_pool(name="xin", bufs=6) as xin,
            tc.tile_pool(name="eo", bufs=3) as eo_pool,
            tc.tile_pool(name="tab", bufs=3) as tab_pool,
            tc.tile_pool(name="qx", bufs=3) as qx_pool,
            tc.tile_pool(name="vat", bufs=4) as vat,
            tc.tile_pool(name="osb", bufs=4) as osb_pool,
            tc.tile_pool(name="pqk", bufs=2, space="PSUM") as pqk,
            tc.tile_pool(name="pattn", bufs=1, space="PSUM") as pattn,
            tc.tile_pool(name="pout", bufs=1, space="PSUM") as pout,
        ):
            # ---- constants ----
            w_sb = [const.tile([128, 768], BF16, name=f"w{h}", tag=f"w{h}")
                    for h in range(2)]
            t_sb = [const.tile([128, 3904], BF16, name=f"t{i}", tag=f"t{i}")
                    for i in range(2)]
            dt_sb = const.tile([128, 18 * 405], BF16, name="dtab", tag="dtab")
            def load_x(t):
                xi = xin.tile([128, 1024], BF16, name="xi", tag="xi")
                nc.sync.dma_start(xi[:], X_d[t])
                return xi

            # DMA issue order matters at kernel start: compute needs only
            # W + X(0..2) to begin; the 4MB of xpos/decay tables queue after
            # (needed ~10us in) so the PE isn't idle behind them.
            for h in range(2):
                nc.sync.dma_start(w_sb[h][:], W_d[h])
            xt_fifo = [load_x(i) for i in range(3)]
            for i in range(2):
                nc.sync.dma_start(t_sb[i][:], T_d[i])
            for s in range(18):
                nc.sync.dma_start(dt_sb[:, s * 405:(s + 1) * 405], DT_d[s])

            for q in range(NQUAD):
                qeo = eo_pool.tile([128, 1952], BF16, name="qeo", tag="qeo")
                keo = eo_pool.tile([128, 1952], BF16, name="keo", tag="keo")
                vsb = [None, None]
                for hh in range(2):
                    t = 2 * q + hh
                    xt = xt_fifo.pop(0)

                    # ---- proj Q,K: psum [128,1024], e block 0:488, o 512:1000
                    pq_t = pqk.tile([128, 1024], F32, name="pq", tag="pqk")
                    pk_t = pqk.tile([128, 1024], F32, name="pk", tag="pqk")
                    for ti, ps in ((0, pq_t), (1, pk_t)):
                        for eo in range(2):
                            for hc in range(2):
                                mov = xt[:, hc * 512:hc * 512 + 512].rearrange(
                                    "p (b l) -> p b l", b=2)[:, :, 0:LP]
                                nc.tensor.matmul(
                                    ps[:, eo * 512:eo * 512 + 488],
                                    w_sb[hc][:, (ti * 2 + eo) * 128:
                                             (ti * 2 + eo) * 128 + 128],
                                    mov,
                                    start=(hc == 0), stop=(hc == 1),
                                )

                    # ---- V: psum [128,1024] = [b0 m0 | b0 m1 | b1 m0 | b1 m1]
                    pv_t = pqk.tile([128, 1024], F32, name="pv", tag="pqk")
                    for b in range(2):
                        for mc in range(2):
                            for hc in range(2):
                                off = hc * 512 + b * 256 + mc * 128
                                nc.tensor.matmul(
                                    pv_t[:, b * 512 + mc * 256:
                                         b * 512 + mc * 256 + 256],
                                    xt[:, off:off + 128],
                                    w_sb[hc][:, 512:768],
                                    start=(hc == 0), stop=(hc == 1),
                                )

                    # prefetch 3 pairs ahead
                    if t + 3 < NPAIR:
                        xt_fifo.append(load_x(t + 3))

                    # ---- ACT drains (psum f32 -> sbuf bf16)
                    src = pq_t[:].rearrange("p (e c) -> p e c", e=2)[:, :, 0:488]
                    dst = qeo[:, hh * 976:hh * 976 + 976].rearrange(
                        "p (e c) -> p e c", e=2)
                    nc.scalar.copy(dst, src)
                    src = pk_t[:].rearrange("p (e c) -> p e c", e=2)[:, :, 0:488]
                    dst = keo[:, hh * 976:hh * 976 + 976].rearrange(
                        "p (e c) -> p e c", e=2)
                    nc.scalar.copy(dst, src)
                    vt = vat.tile([128, 1024], BF16, name="vsb", tag=f"vsb{hh}")
                    nc.scalar.copy(vt[:], pv_t[:])
                    vsb[hh] = vt

                # ---- xpos muls (DVE, bf16 2x): one fused op per side reads
                # the projections twice via a stride-0 broadcast dim and
                # produces all four products [TA_p0|TB_p0|TA_p1|TB_p1]
                ta_q = tab_pool.tile([128, 3904], BF16, name="taq", tag="taq")
                ta_k = tab_pool.tile([128, 3904], BF16, name="tak", tag="tak")
                for src_eo, tab, dst in ((qeo, t_sb[0], ta_q),
                                         (keo, t_sb[1], ta_k)):
                    s = src_eo[:].rearrange("p (pr c) -> p pr c", pr=2)
                    s = s.unsqueeze(2).broadcast_to([128, 2, 2, 976])
                    nc.vector.tensor_tensor(
                        dst[:].rearrange("p (pr r c) -> p pr r c", pr=2, r=2),
                        s,
                        tab[:].rearrange("p (pr r c) -> p pr r c", pr=2, r=2),
                        MULT)

                # ---- combines -> qx/kx [128,1024] 256-strided (pads stay 0)
                qx_e = qx_pool.tile([128, 1024], BF16, name="qxe", tag="qxe")
                qx_o = qx_pool.tile([128, 1024], BF16, name="qxo", tag="qxo")
                kx_e = qx_pool.tile([128, 1024], BF16, name="kxe", tag="kxe")
                kx_o = qx_pool.tile([128, 1024], BF16, name="kxo", tag="kxo")
                # (no pad zeroing needed: S mt1 stationary reads only the 115
                # real columns, and qx is only read as moving over real cols)

                def c_src(tab, blk):
                    # 488-blocks per pair: 0=QeC, 1=QoS, 2=QeS, 3=QoC
                    a = tab[:].rearrange("p (pr c) -> p pr c", pr=2)
                    a = a[:, :, blk * 488:(blk + 1) * 488]
                    return a.rearrange("p pr (b l) -> p pr b l", b=2)

                def c_dst(dst):
                    return dst[:].rearrange(
                        "p (pr b l) -> p pr b l", pr=2, b=2)[:, :, :, 0:LP]

                # all combines on DVE: gpsimd elementwise halves DVE
                # throughput via the shared SBUF port (measured)
                nc.vector.tensor_tensor(
                    c_dst(qx_e), c_src(ta_q, 0), c_src(ta_q, 1), SUB)
                nc.vector.tensor_tensor(
                    c_dst(qx_o), c_src(ta_q, 3), c_src(ta_q, 2), ADD)
                nc.vector.tensor_tensor(
                    c_dst(kx_e), c_src(ta_k, 0), c_src(ta_k, 1), SUB)
                nc.vector.tensor_tensor(
                    c_dst(kx_o), c_src(ta_k, 3), c_src(ta_k, 2), ADD)

                # ---- attention per pair ----
                for hh in range(2):
                    t = 2 * q + hh
                    osb = osb_pool.tile([128, 972], BF16,
                                        name="ob", tag=f"ob{hh}")
                    po = pout.tile([128, 1024], F32, name="ops", tag="outp")
                    # scores for both batches in one 2-bank tile:
                    # batch bl at cols bl*512 + [mt0 0:243 | mt1 243:405]
                    ps = pattn.tile([128, 1024], F32, name="sps", tag="attn")
                    for bl in range(2):
                        boff = (2 * hh + bl) * 256
                        soff = bl * 512
                        nc.tensor.matmul(ps[:, soff:soff + 243],
                                         kx_e[:, boff:boff + 128],
                                         qx_e[:, boff:boff + 243],
                                         start=True, stop=False)
                        nc.tensor.matmul(ps[:, soff:soff + 243],
                                         kx_o[:, boff:boff + 128],
                                         qx_o[:, boff:boff + 243],
                                         start=False, stop=True)
                        nc.tensor.matmul(ps[0:115, soff + 243:soff + 405],
                                         kx_e[:, boff + 128:boff + 243],
                                         qx_e[:, boff + 81:boff + 243],
                                         start=True, stop=False)
                        nc.tensor.matmul(ps[0:115, soff + 243:soff + 405],
                                         kx_o[:, boff + 128:boff + 243],
                                         qx_o[:, boff + 81:boff + 243],
                                         start=False, stop=True)

                    # decay mask for the pair (DVE, psum 1x) -> A^T bf16;
                    # slots (2t)%17 and +1 are contiguous in the 18-slot table
                    s0 = (2 * t) % J
                    at = vat.tile([128, 832], BF16, name="at", tag=f"at{hh}")
                    nc.vector.tensor_tensor(
                        at[:].rearrange("p (k c) -> p k c", k=2)[:, :, 0:405],
                        ps[:].rearrange("p (k c) -> p k c", k=2)[:, :, 0:405],
                        dt_sb[:, s0 * 405:s0 * 405 + 810].rearrange(
                            "p (k c) -> p k c", k=2),
                        MULT)

                    # AV: out^T [128, 486] = [ht0 l 0:243 | ht1 l 0:243]
                    v = vsb[hh]
                    for bl in range(2):
                        aoff = bl * 416
                        for ht in range(2):
                            lhs0 = v[:, bl * 512 + ht * 128:
                                     bl * 512 + ht * 128 + 128]
                            # mt1: only 115 real m rows — rows 115:128 of the
                            # S/A mt1 region are never written and never read
                            lhs1 = v[0:115, bl * 512 + 256 + ht * 128:
                                     bl * 512 + 256 + ht * 128 + 128]
                            base = bl * 512 + ht * 243
                            nc.tensor.matmul(po[:, base:base + 243],
                                             lhs0, at[:, aoff:aoff + 243],
                                             start=True, stop=False)
                            nc.tensor.matmul(po[:, base + 81:base + 243],
                                             lhs1, at[0:115, aoff + 243:aoff + 405],
                                             start=False, stop=True)

                    # one merged out drain per pair on ACT
                    src = po[:].rearrange("p (b c) -> p b c", b=2)[:, :, 0:486]
                    dst = osb[:].rearrange("p (b c) -> p b c", b=2)
                    nc.scalar.copy(dst, src)
                    nc.sync.dma_start(O_d[t], osb[:])

    nc.compile()
    return nc


def _get_nc():
    if "nc" not in _cache:
        _cache["nc"] = _build()
    return _cache["nc"]


def _run(in_maps, trace=False):
    from concourse import bass_utils
    nc = _get_nc()
    return bass_utils.run_bass_kernel_spmd(
        nc, in_maps, core_ids=list(range(NCORES)), trace=trace)


def kernel(X, W_Q, W_K, W_V, gamma, _trace=False):
    X = np.asarray(X, f32)
    W_all, T_all, DTab = _host_tables(
        np.asarray(W_Q, f32), np.asarray(W_K, f32),
        np.asarray(W_V, f32), np.asarray(gamma, f32))

    in_maps = []
    for c in range(NCORES):
        in_maps.append({
            "X": _host_pack_x(X[c * BPC:(c + 1) * BPC]),
            "WALL": W_all, "TTAB": T_all, "DTAB": DTab,
        })
    res = _run(in_maps, trace=_trace)
    out = np.concatenate([_host_unpack_o(r["OUT"]) for r in res.results],
                         axis=0)
    if _trace:
        _cache["last_result"] = res
    return out.astype(f32)
